# revision 11
# baseline (speedup 1.0000x reference)
"""Balanced BCE loss on 8 Trainium2 NeuronCores.

loss = -sum_i [ beta_i * sum_j(t_ij * ln(p_ij))
                + (1-beta_i) * sum_j((1-t_ij) * ln(1-p_ij)) ]
beta_i = 1 - mean_j(t_ij)

Per-core slab layout: the core's 8 rows (8MB contiguous HBM) are viewed
as [128, 16384] f32 where partition p owns the contiguous 64KB line
slab[p*16384:(p+1)*16384]; row r <-> partitions 16r..16r+15.

Streaming per column-chunk C (all reductions fused into the producers,
no per-chunk matmuls):
  ACT: l1mp = Ln(1-p) bf16 with accum_out -> accB[:,c]
       logp = Ln(p)  bf16
  DVE: tensor_scalar   tb = bf16(t)        accum_out -> accS[:,c]
       tensor_tensor_reduce  tb*l1mp       accum_out -> accC[:,c]
       tensor_tensor_reduce  tb*logp       accum_out -> accA[:,c]

Epilogue: 4 tiny free-dim reduces -> acc4[128,4]; DMA out [128,4];
host folds each row's 16 partitions and combines:
loss = -sum_rows[ beta*A + (1-beta)*(B-C) ], beta = 1-S/N

DMA: p-chunks on the SP HWDGE ring (nc.sync), t-chunks on the ACT
HWDGE ring (nc.scalar), interleaved, big chunks mid-stream and small
chunks last so the compute tail after the final transfer is short.
"""

from contextlib import ExitStack

import numpy as np

import concourse.bass as bass
import concourse.mybir as mybir
import concourse.tile as tile
from concourse import bacc
from concourse.bass_utils import run_bass_kernel_spmd

B, N = 64, 262144
NCORES = 8
ROWS = B // NCORES  # rows per core
P = 128  # SBUF partitions
F = ROWS * N // P  # 16384 cols per partition
GRP = P // ROWS  # 16 partitions per row

# column-chunk schedule: sums to F; mid-stream big (2MB DMAs), tail small
CHUNKS = [2048, 4096, 4096, 2048, 2048, 1024, 1024]
assert sum(CHUNKS) == F
NCH = len(CHUNKS)
CMAX = max(CHUNKS)

AF = mybir.ActivationFunctionType
ALU = mybir.AluOpType
f32 = mybir.dt.float32
bf16 = mybir.dt.bfloat16

# test.py can flip this to capture an NTFF profile of the run
TRACE = False
LAST = None  # BassKernelResults of the most recent kernel() call


def _emit(tc, out_ap, inp_ap, tgt_ap):
    nc = tc.nc

    with ExitStack() as ctx:
        singles = ctx.enter_context(tc.tile_pool(name="const", bufs=1))
        bf_pool = ctx.enter_context(tc.tile_pool(name="bf", bufs=2))

        pslab = singles.tile([P, F], f32, tag="pslab")
        tslab = singles.tile([P, F], f32, tag="tslab")
        tb = singles.tile([P, CMAX], bf16, tag="tb")
        junk = singles.tile([P, CMAX], bf16, tag="junk")
        accS = singles.tile([P, NCH], f32, tag="accS")
        accB = singles.tile([P, NCH], f32, tag="accB")
        accA = singles.tile([P, NCH], f32, tag="accA")
        accC = singles.tile([P, NCH], f32, tag="accC")
        acc4 = singles.tile([P, 4], f32, tag="acc4")

        # slab views: [rows, n] -> [128, F], 64KB contiguous per partition
        inp3 = inp_ap.rearrange("r (a f) -> (r a) f", a=GRP)
        tgt3 = tgt_ap.rearrange("r (a f) -> (r a) f", a=GRP)

        # interleave p/t chunk triggers; p on SP ring, t on ACT ring
        offs = [0]
        for c in CHUNKS:
            offs.append(offs[-1] + c)
        for c in range(NCH):
            o, e = offs[c], offs[c + 1]
            nc.sync.dma_start(pslab[:, o:e], inp3[:, o:e])
            nc.sync.dma_start(tslab[:, o:e], tgt3[:, o:e])

        for c in range(NCH):
            o, e = offs[c], offs[c + 1]
            w = e - o
            p_t = pslab[:, o:e]
            t_t = tslab[:, o:e]

            l1mp = bf_pool.tile([P, CMAX], bf16, tag="l1mp")
            nc.scalar.activation(
                l1mp[:, :w], p_t, AF.Ln, scale=-1.0, bias=1.0,
                accum_out=accB[:, c : c + 1],
            )
            logp = bf_pool.tile([P, CMAX], bf16, tag="logp")
            nc.scalar.activation(logp[:, :w], p_t, AF.Ln)

            # cast + per-partition sum(t) in one op (accum_out needs a
            # real op1, so mult-by-1 then add-0)
            nc.vector.tensor_scalar(
                tb[:, :w], t_t, 1.0, 0.0, ALU.mult, ALU.add,
                accum_out=accS[:, c : c + 1],
            )
            # product + per-partition sum in one op
            # (tensor_tensor_reduce crashes the exec unit on this HW;
            # scalar_tensor_tensor with mult-by-1 is equivalent)
            nc.vector.scalar_tensor_tensor(
                junk[:, :w], tb[:, :w], 1.0, l1mp[:, :w],
                op0=ALU.mult, op1=ALU.mult,
                accum_out=accC[:, c : c + 1],
            )
            nc.vector.scalar_tensor_tensor(
                junk[:, :w], tb[:, :w], 1.0, logp[:, :w],
                op0=ALU.mult, op1=ALU.mult,
                accum_out=accA[:, c : c + 1],
            )

        # fold the per-chunk columns; the cross-partition (per-row)
        # reduction of the remaining 128x4 values happens on the host
        for s, acc in enumerate((accS, accB, accA, accC)):
            nc.vector.tensor_reduce(
                acc4[:, s : s + 1], acc[:], axis=mybir.AxisListType.X, op=ALU.add
            )
        nc.sync.dma_start(out_ap, acc4[:])


_PROG_CACHE = {}


def _build_program():
    key = "v2"
    if key not in _PROG_CACHE:
        nc = bacc.Bacc("TRN2", target_bir_lowering=False, debug=False)
        inp = nc.dram_tensor("input", [ROWS, N], f32, kind="ExternalInput").ap()
        tgt = nc.dram_tensor("target", [ROWS, N], f32, kind="ExternalInput").ap()
        out = nc.dram_tensor("partials", [P, 4], f32, kind="ExternalOutput").ap()
        with tile.TileContext(nc) as tc:
            _emit(tc, out, inp, tgt)
        nc.finalize()
        _PROG_CACHE[key] = nc
    return _PROG_CACHE[key]


def kernel(input, target):
    global LAST
    input = np.ascontiguousarray(np.asarray(input))
    target = np.ascontiguousarray(np.asarray(target))
    assert input.shape == (B, N) and target.shape == (B, N)

    nc = _build_program()
    in_maps = [
        {
            "input": input[c * ROWS : (c + 1) * ROWS],
            "target": target[c * ROWS : (c + 1) * ROWS],
        }
        for c in range(NCORES)
    ]
    res = run_bass_kernel_spmd(nc, in_maps, core_ids=list(range(NCORES)), trace=TRACE)
    LAST = res

    total = np.float64(0.0)
    for c in range(NCORES):
        part = res.results[c]["partials"].astype(np.float64)  # [128, 4]
        part = part.reshape(ROWS, GRP, 4).sum(axis=1)  # [ROWS, 4]
        S, Bv, A, C = part[:, 0], part[:, 1], part[:, 2], part[:, 3]
        beta = 1.0 - S / N
        total += np.sum(beta * A + (1.0 - beta) * (Bv - C))
    return np.float32(-total)


# revision 14
# speedup vs baseline: 1.2719x; 1.2719x over previous
"""Balanced BCE loss on 8 Trainium2 NeuronCores.

loss = -sum_i [ beta_i * sum_j(t_ij * ln(p_ij))
                + (1-beta_i) * sum_j((1-t_ij) * ln(1-p_ij)) ]
beta_i = 1 - mean_j(t_ij)

Per-core slab layout: the core's 8 rows (8MB contiguous HBM) are viewed
as [128, 16384] f32 where partition p owns the contiguous 64KB line
slab[p*16384:(p+1)*16384]; row r <-> partitions 16r..16r+15.

Two independent DMA streams interleave at the SDMA engines, covering
each other's inter-transfer bubbles:
  p-chunks: HWDGE (nc.sync), f32
  t-chunks: SWDGE (nc.gpsimd) with f32->bf16 cast in the DMA datapath
            (kills the DVE cast op and halves t's SBUF footprint)

Streaming per column-chunk (DVE ops all run 2x bf16 mode; the fused
accum-reduce DVE variants only have 1x microcode so reductions go to
the otherwise-idle PE instead):
  ACT: l1mp = Ln(1-p) bf16, accum_out -> accB[:,c]   (B for free)
       logp = Ln(p)  bf16
  DVE: m2 = t*l1mp, m1 = t*logp   (plain tensor_tensor, 2x)
  PE:  E^T @ {t, m2, m1} in 512-col sub-blocks accumulated into
       psS/psC/psA [8,512] PSUM across all chunks (E = block-indicator
       [128,8], host-provided constant; E[p,r]=1 iff p//16==r)

Epilogue: accB -> accBr [128,1] (DVE), psB = E^T @ accBr (PE);
psS/psA folded on ACT (Copy+accum reads PSUM), psC on DVE; stats [8,4]
DMA'd out; host: loss = -sum_rows[ beta*A + (1-beta)*(B-C) ].
"""

from contextlib import ExitStack

import numpy as np

import concourse.bass as bass
import concourse.mybir as mybir
import concourse.tile as tile
from concourse import bacc
from concourse.bass_utils import run_bass_kernel_spmd

B, N = 64, 262144
NCORES = 8
ROWS = B // NCORES  # rows per core
P = 128  # SBUF partitions
F = ROWS * N // P  # 16384 cols per partition
GRP = P // ROWS  # 16 partitions per row

# column-chunk schedule: sums to F; mid-stream big (2MB p-DMAs), tail small
CHUNKS = [2048, 4096, 4096, 2048, 2048, 1024, 1024]
assert sum(CHUNKS) == F
NCH = len(CHUNKS)
CMAX = max(CHUNKS)
MM = 512  # matmul sub-block width (one PSUM bank)

AF = mybir.ActivationFunctionType
ALU = mybir.AluOpType
f32 = mybir.dt.float32
bf16 = mybir.dt.bfloat16

# test.py can flip this to capture an NTFF profile of the run
TRACE = False
LAST = None  # BassKernelResults of the most recent kernel() call


def _emit(tc, out_ap, inp_ap, tgt_ap, emat_ap):
    nc = tc.nc

    with ExitStack() as ctx:
        singles = ctx.enter_context(tc.tile_pool(name="const", bufs=1))
        pch_pool = ctx.enter_context(tc.tile_pool(name="pch", bufs=3))
        tch_pool = ctx.enter_context(tc.tile_pool(name="tch", bufs=3))
        ln_pool = ctx.enter_context(tc.tile_pool(name="ln", bufs=2))
        mm_pool = ctx.enter_context(tc.tile_pool(name="mm", bufs=2))
        psum_pool = ctx.enter_context(tc.tile_pool(name="ps", bufs=1, space="PSUM"))

        accB = singles.tile([P, NCH], f32, tag="accB")
        junkps = singles.tile([ROWS, MM], bf16, tag="junkps")
        accBr = singles.tile([P, 1], f32, tag="accBr")
        ematf = singles.tile([P, ROWS], f32, tag="ematf")
        ematb = singles.tile([P, ROWS], bf16, tag="ematb")
        stats = singles.tile([ROWS, 4], f32, tag="stats")
        psS = psum_pool.tile([ROWS, MM], f32, tag="psS", name="psS")
        psA = psum_pool.tile([ROWS, MM], f32, tag="psA", name="psA")
        psC = psum_pool.tile([ROWS, MM], f32, tag="psC", name="psC")
        psB = psum_pool.tile([ROWS, 1], f32, tag="psB", name="psB")

        # slab views: [rows, n] -> [128, F], 64KB contiguous per partition
        inp3 = inp_ap.rearrange("r (a f) -> (r a) f", a=GRP)
        tgt3 = tgt_ap.rearrange("r (a f) -> (r a) f", a=GRP)

        offs = [0]
        for c in CHUNKS:
            offs.append(offs[-1] + c)

        # all DMA triggers upfront: p on the SP HWDGE ring, t via SWDGE
        # with inline f32->bf16 cast; the two queues round-robin at the
        # SDMA engines so one stream's boundary stalls hide in the other
        ptiles, ttiles = [], []
        for c in range(NCH):
            o, e = offs[c], offs[c + 1]
            pt = pch_pool.tile([P, CMAX], f32, tag="p", name=f"p{c}")
            nc.sync.dma_start(pt[:, : e - o], inp3[:, o:e])
            ptiles.append(pt)
            tt = tch_pool.tile([P, CMAX], bf16, tag="t", name=f"t{c}")
            nc.gpsimd.dma_start(tt[:, : e - o], tgt3[:, o:e])
            ttiles.append(tt)
            if c == 0:
                nc.sync.dma_start(ematf[:], emat_ap)

        nc.vector.tensor_copy(ematb[:], ematf[:])

        nblk = 0
        nblk_total = F // MM
        for c in range(NCH):
            w = CHUNKS[c]
            p_t = ptiles[c][:, :w]
            t_t = ttiles[c][:, :w]

            l1mp = ln_pool.tile([P, CMAX], bf16, tag="l1mp")
            nc.scalar.activation(
                l1mp[:, :w], p_t, AF.Ln, scale=-1.0, bias=1.0,
                accum_out=accB[:, c : c + 1],
            )
            logp = ln_pool.tile([P, CMAX], bf16, tag="logp")
            nc.scalar.activation(logp[:, :w], p_t, AF.Ln)

            m2 = mm_pool.tile([P, CMAX], bf16, tag="m2")
            nc.vector.tensor_mul(m2[:, :w], t_t, l1mp[:, :w])
            m1 = mm_pool.tile([P, CMAX], bf16, tag="m1")
            nc.vector.tensor_mul(m1[:, :w], t_t, logp[:, :w])

            for b in range(w // MM):
                s, e = b * MM, (b + 1) * MM
                first, last = nblk == 0, nblk == nblk_total - 1
                nc.tensor.matmul(psS[:], ematb[:], t_t[:, s:e], start=first, stop=last)
                nc.tensor.matmul(psC[:], ematb[:], m2[:, s:e], start=first, stop=last)
                nc.tensor.matmul(psA[:], ematb[:], m1[:, s:e], start=first, stop=last)
                nblk += 1

        # epilogue: B row-sums via one tiny f32 matmul; PSUM folds split
        # between ACT (reads PSUM cheaply) and DVE so they overlap
        nc.vector.tensor_reduce(accBr[:], accB[:], axis=mybir.AxisListType.X, op=ALU.add)
        nc.tensor.matmul(psB[:], ematf[:], accBr[:])
        nc.scalar.activation(junkps[:], psS[:], AF.Copy, accum_out=stats[:, 0:1])
        nc.scalar.activation(junkps[:], psA[:], AF.Copy, accum_out=stats[:, 2:3])
        nc.vector.tensor_reduce(stats[:, 3:4], psC[:], axis=mybir.AxisListType.X, op=ALU.add)
        nc.vector.tensor_copy(stats[:, 1:2], psB[:])
        nc.sync.dma_start(out_ap, stats[:])


_PROG_CACHE = {}


def _build_program():
    key = "v4"
    if key not in _PROG_CACHE:
        nc = bacc.Bacc("TRN2", target_bir_lowering=False, debug=False)
        inp = nc.dram_tensor("input", [ROWS, N], f32, kind="ExternalInput").ap()
        tgt = nc.dram_tensor("target", [ROWS, N], f32, kind="ExternalInput").ap()
        emat = nc.dram_tensor("emat", [P, ROWS], f32, kind="ExternalInput").ap()
        out = nc.dram_tensor("partials", [ROWS, 4], f32, kind="ExternalOutput").ap()
        with tile.TileContext(nc) as tc:
            _emit(tc, out, inp, tgt, emat)
        nc.finalize()
        _PROG_CACHE[key] = nc
    return _PROG_CACHE[key]


def _emat_np():
    e = np.zeros((P, ROWS), dtype=np.float32)
    for r in range(ROWS):
        e[r * GRP : (r + 1) * GRP, r] = 1.0
    return e


def kernel(input, target):
    global LAST
    input = np.ascontiguousarray(np.asarray(input))
    target = np.ascontiguousarray(np.asarray(target))
    assert input.shape == (B, N) and target.shape == (B, N)

    nc = _build_program()
    emat = _emat_np()
    in_maps = [
        {
            "input": input[c * ROWS : (c + 1) * ROWS],
            "target": target[c * ROWS : (c + 1) * ROWS],
            "emat": emat,
        }
        for c in range(NCORES)
    ]
    res = run_bass_kernel_spmd(nc, in_maps, core_ids=list(range(NCORES)), trace=TRACE)
    LAST = res

    total = np.float64(0.0)
    for c in range(NCORES):
        part = res.results[c]["partials"].astype(np.float64)  # [ROWS, 4]
        S, Bv, A, C = part[:, 0], part[:, 1], part[:, 2], part[:, 3]
        beta = 1.0 - S / N
        total += np.sum(beta * A + (1.0 - beta) * (Bv - C))
    return np.float32(-total)
